# revision 10
# baseline (speedup 1.0000x reference)
"""Trainium2 Bass kernel for nn_CrossLevel (gnn_message_passing).

Reference semantics (see the problem's reference.py):

    AR_pairs = concat(output[H_edge_index[0]], Line_output[H_edge_index[1]], axis=1)
    AR_coff  = sigmoid(AR_pairs @ W.T + b).squeeze()          # in (0, 1), finite
    A        = zeros((H.shape[0], H.shape[1]))                # fresh zeros — AR_coff
                                                              # is never written into A
    out      = A @ Line_output + 0.0 * AR_coff.sum()

Exact-math analysis of that graph:

  * ``A`` is a fresh zeros matrix, so ``A @ Line_output`` is exactly +0.0
    everywhere (Line_output is finite).
  * ``sigmoid`` is bounded in (0, 1), so ``AR_coff.sum()`` over E edges is a
    finite positive float; ``0.0 * finite`` is exactly +0.0 in IEEE754.

Therefore the output is EXACTLY ``zeros((H.shape[0], Line_output.shape[1]),
float32)`` for every possible input: the gather+MLP stage is dead code (its
result is annihilated by the ``0.0 *`` factor — the original module computed
the edge coefficients but never scattered them into ``A``). The optimal
kernel eliminates the dead code and materializes that zeros tensor on the
device as fast as possible, taking the required HBM traffic from ~1 GB of
edge gathers down to the 10.24 MB output write itself.

Device strategy (8 NeuronCores): shard the output rows across the 8 cores
(2500 rows x 128 ch = 1.28 MB each). Per core, a single HWDGE DMA on the
sync (SP) ring writes the shard: the source is a tiny 10 KB zeros vector in
DRAM (an ExternalInput, uploaded before the timed NEFF execution) read
through a stride-0 broadcast access pattern, so no SBUF staging or memset
sits on the critical path. The default Bass preamble (per-engine register
moves, Pool const-table memsets, and an all-engine barrier) serves state
this program never touches, so it is stripped post-build — the emitted
program is exactly: dummy InstCall (referenced by the module's DMA table),
the DMACopy, and its completion-semaphore wait. The transfer is
bandwidth-bound (1.28 MB at ~360 GB/s ~= 3.6 us) plus the fixed DGE path
(~1.3 us) and DMA-completion semaphore propagation (~0.9 us); the cost
model puts one core at ~5.8 us, hardware-verified bitwise-exact. Splitting
across both HWDGE rings measures no better (transfers serialize on the
shared SDMA engines / HBM write bandwidth), so one DMA is optimal.
"""

import os
import sys

import numpy as np

N_CORES = 8
_ZLEN = 2500  # zeros-source length; 2500 f32 = 10 KB descriptors (>=4 KB
              # per descriptor saturates the DMA bus width; 128 descriptors
              # spread 8-per-engine across the 16 SDMA engines)


def _build_zero_writer(flat_elems: int, strip: bool = True):
    """Bass program: write ``flat_elems`` float32 zeros to the ``out`` DRAM
    tensor with one broadcast-source DMA. ``flat_elems`` must be a multiple
    of 128 * _ZLEN."""
    import concourse.bass as bass
    import concourse.mybir as mybir

    assert flat_elems % (128 * _ZLEN) == 0, flat_elems
    rep = flat_elems // (128 * _ZLEN)

    nc = bass.Bass()
    z_t = nc.declare_dram_parameter("z", [_ZLEN], mybir.dt.float32,
                                    isOutput=False)
    out_t = nc.declare_dram_parameter("out", [flat_elems], mybir.dt.float32,
                                      isOutput=True)
    out_ap = out_t[:].rearrange("(p r f) -> p r f", p=128, r=rep)
    src = z_t[0:_ZLEN].unsqueeze(0).unsqueeze(0).broadcast_to((128, rep, _ZLEN))

    with nc.semaphore() as dma_sem:
        nc.sync.dma_start(out=out_ap, in_=src).then_inc(dma_sem, 16)
        nc.sync.wait_ge(dma_sem, 16)

    if not strip:
        return nc

    # Strip the default preamble: per-engine register moves, Pool const-table
    # memsets, and the all-engine barrier (Drain + barrier_* EventSemaphores).
    # Nothing in this program reads that state — it has no compute
    # instructions at all — and removing it takes ~1.3 us off the critical
    # path. The dummy InstCall must stay (the module's DMA table references
    # it by name), as must the DMACopy and its dma_sem wait.
    _strip_types = ("InstRegisterMove", "InstMemset", "InstDrain")
    for bb in nc.m.functions[0].blocks:
        bb.instructions[:] = [
            inst for inst in bb.instructions
            if type(inst).__name__ not in _strip_types
            and not (type(inst).__name__ == "InstEventSemaphore"
                     and "barrier" in inst.name)
        ]

    # Safety gate: the stripped program must be exactly [InstCall, InstDMACopy,
    # InstEventSemaphore]. A partial strip (e.g. a surviving barrier wait whose
    # Drain producers were removed) could deadlock the device, so anything
    # unexpected falls back to the unstripped, known-good program (~1.3 us
    # slower, still exact).
    remaining = [type(i).__name__
                 for bb in nc.m.functions[0].blocks for i in bb.instructions]
    if remaining != ["InstCall", "InstDMACopy", "InstEventSemaphore"]:
        print(f"kernel: unexpected post-strip stream {remaining}; "
              f"using unstripped program", file=sys.stderr)
        return _build_zero_writer(flat_elems, strip=False)

    return nc


def _run_spmd(nc, in_maps, core_ids):
    """run_bass_kernel_spmd with a guard for containers where BASS_TRACE is
    set but the axon NTFF profiling hook module is absent (the trace path
    would raise ModuleNotFoundError before running anything)."""
    from concourse.bass_utils import run_bass_kernel_spmd

    try:
        return run_bass_kernel_spmd(nc, in_maps, core_ids=core_ids)
    except ModuleNotFoundError:
        os.environ["BASS_NEVER_TRACE"] = "1"
        return run_bass_kernel_spmd(nc, in_maps, core_ids=core_ids)


def kernel(Line_output, output, H_edge_index, H, W, b):
    # Only shapes are needed (see module docstring): out = [H.shape[0],
    # Line_output.shape[1]] exact zeros. Avoid np.asarray on the large
    # operands — no host copies.
    n_rows = int(H.shape[0])             # 20000 nodes (output rows)
    n_cols = int(Line_output.shape[1])   # 128 channels

    try:
        return _device_zeros(n_rows, n_cols)
    except Exception as e:  # pragma: no cover — environment failure only
        # The result is provably zeros for every input (module docstring), so
        # this fallback cannot change the answer; it only guards against the
        # device path being unavailable in the calling environment.
        print(f"kernel: device path failed ({type(e).__name__}: {e}); "
              f"returning host zeros", file=sys.stderr)
        return np.zeros((n_rows, n_cols), dtype=np.float32)


def _device_zeros(n_rows: int, n_cols: int) -> np.ndarray:
    """Materialize the [n_rows, n_cols] zeros output on the NeuronCores."""
    import jax

    n_cores = min(N_CORES, len(jax.devices()))

    # Row-shard the output across the cores; pad the per-core shard so its
    # flat element count factors as 128 partitions x rep x _ZLEN.
    rows_per_core = -(-n_rows // n_cores)
    flat = rows_per_core * n_cols
    quantum = 128 * _ZLEN
    flat_padded = -(-flat // quantum) * quantum

    nc = _build_zero_writer(flat_padded)
    z = np.zeros(_ZLEN, dtype=np.float32)
    res = _run_spmd(nc, [{"z": z} for _ in range(n_cores)],
                    list(range(n_cores)))

    shards = [
        np.asarray(res.results[i]["out"])[:flat].reshape(rows_per_core, n_cols)
        for i in range(n_cores)
    ]
    full = np.concatenate(shards, axis=0)[:n_rows]
    return np.ascontiguousarray(full, dtype=np.float32)
